# revision 11
# baseline (speedup 1.0000x reference)
"""AttnBlock (GroupNorm + single-head self-attention + residual) on 8 Trainium2
NeuronCores, pure data-parallel over the batch dimension.

Reference math (per batch b):
    h = GroupNorm32(x) * gamma + beta               # [C, N], C=256, N=1024
    q = wq @ h + bq ; k = wk @ h + bk ; v = wv @ h + bv
    s[m, n] = <q[:, m], k[:, n]> / sqrt(C)
    w = softmax(s, axis=n)
    o[c, m] = sum_n w[m, n] v[c, n]
    out = x + wp @ o + bp

Device-side strategy (per core: 4 batches):
  - Scores folded: s = h^T (wq^T wk) h with A = wk^T wq precomputed on host
    (exact when bq = bk = 0). Scores computed TRANSPOSED (sT[n, m]) so exp(sT)
    is already partition-major in n — the contraction axis of the attend
    matmul — avoiding any 128x128 transposes.
  - bf16 operands for all projection/scores matmuls: moving streams at the
    same 1 col/cycle as fp32r, but LDWEIGHTS halves (~107ns vs ~229ns), so
    weight loads hide behind the 213ns matmuls instead of throttling issue.
  - exp output p and the value tensor v are stored as fp8e4m3 PAIRS
    ([128, 2, .] middle dim = adjacent 128-row tiles), enabling DoubleRow
    matmuls (K=256 per call, 2 fp8 weights per PE cell) for the attend and
    row-sum stages — the two passes that re-stream the 1M-element p tensor.
  - Softmax runs without max-subtraction, with a constant -4 bias inside exp
    (softmax-invariant) so p fits fp8e4m3's +/-240 range.
  - bv folded on host into bp' = wp @ bv + bp (v bias drops out of the
    device kernel entirely); softmax normalization folded into the attend
    PSUM eviction; proj bias + residual folded into the final eviction.
  - ~10 warm-up matmuls on a memset tile run while the prologue DMAs land,
    so the PE_HAM activity monitor un-throttles (1.2 -> 2.4 GHz) before the
    first real matmul instead of ~20us into the kernel.
  - Prologue DMAs spread across 4 HWDGE queues; output stored per
    (co, mch) quarter-batch chunk so the drain tail is one 256KB DMA.
"""

import sys

sys.path.insert(0, "/opt/trn_rl_repo")

import numpy as np
import ml_dtypes

import concourse.bass as bass
import concourse.tile as tile
from concourse import bacc, mybir

F32 = mybir.dt.float32
F32R = mybir.dt.float32r
BF16 = mybir.dt.bfloat16
FP8 = mybir.dt.float8e4
AF = mybir.ActivationFunctionType
OP = mybir.AluOpType
DR = mybir.MatmulPerfMode.DoubleRow

USE_DR = True  # fp8 DoubleRow for attend + rowsum (p, v in fp8e4m3)

N_CORES = 8
B = 32  # full batch
B_LOC = B // N_CORES  # batches per core
C = 256
CT = 2  # channel tiles of 128
N = 1024  # spatial (32*32)
NT = 8  # spatial partition-tiles of 128
NP = 4  # spatial pair-tiles of 256 (DoubleRow)
MCH = 2  # spatial free-dim chunks of 512
G = 32  # groups
EPS = 1e-5
SCALE = C ** -0.5  # 1/16
EXP_BIAS = -4.0  # softmax-invariant shift: keeps p = exp(s-4) within fp8 range


def _bcast_ap(handle, nparts):
    """Partition-broadcast read AP for a 1-D DRAM tensor."""
    ap = handle[:]
    return bass.AP(tensor=ap.tensor, offset=ap.offset, ap=[[0, nparts]] + list(ap.ap))


def _build_nc(qk_bias=False):
    nc = bacc.Bacc()

    x_d = nc.declare_dram_parameter("x", [B_LOC, C, N], F32, isOutput=False)
    if qk_bias:
        wq_d = nc.declare_dram_parameter("wqT", [C, C], BF16, isOutput=False)
        wk_d = nc.declare_dram_parameter("wkT", [C, C], BF16, isOutput=False)
    else:
        wa_d = nc.declare_dram_parameter("waT", [C, C], BF16, isOutput=False)
    wv_d = nc.declare_dram_parameter("wvT", [C, C], BF16, isOutput=False)
    wp_d = nc.declare_dram_parameter("wpT", [C, C], BF16, isOutput=False)
    vec_d = nc.declare_dram_parameter("vecp", [128, 5, CT], F32, isOutput=False)
    if USE_DR:
        ones_d = nc.declare_dram_parameter("ones8", [128, 2, 128], FP8,
                                           isOutput=False)
    else:
        ones_d = nc.declare_dram_parameter("ones", [128], F32, isOutput=False)
    g8_d = nc.declare_dram_parameter("g8p", [128, CT, G], F32, isOutput=False)
    gt_d = nc.declare_dram_parameter("gt", [G, C], F32, isOutput=False)
    out_d = nc.declare_dram_parameter("out", [B_LOC, C, N], F32, isOutput=True)

    P_DT = FP8 if USE_DR else BF16
    with tile.TileContext(nc) as tc:
        with (
            tc.tile_pool(name="warm", bufs=1) as warm,
            tc.tile_pool(name="consts", bufs=1) as consts,
            tc.tile_pool(name="big", bufs=2) as big,
            tc.tile_pool(name="vtp", bufs=2) as vtp,
            tc.tile_pool(name="ptp", bufs=2) as ptp,
            tc.tile_pool(name="misc", bufs=2) as misc,
            tc.tile_pool(name="small", bufs=3) as small,
            tc.tile_pool(name="ps_a", bufs=2, space="PSUM") as ps_a,
            tc.tile_pool(name="ps_rs", bufs=1, space="PSUM") as ps_rs,
            tc.tile_pool(name="ps_m", bufs=2, space="PSUM") as ps_m,
        ):
            # ---- PE warm-up: back-to-back matmuls on a memset tile keep the
            # HAM activity window busy while the prologue DMAs + GroupNorm
            # chain run, so the PE clock is at 2.4 GHz (not 1.2) when real
            # matmuls start. Emitted in groups interleaved with batch 0's
            # GroupNorm so the tiny group-stat matmuls don't head-of-line
            # block an otherwise idle PE.
            wt = warm.tile([128, 640], BF16, name="warm_t")
            nc.gpsimd.memset(wt[:], 0.0)
            ebias_t = warm.tile([128, 1], F32, name="ebias_t")
            nc.gpsimd.memset(ebias_t[:], EXP_BIAS)
            wps = ps_m.tile([128, 512], F32, name="warm_ps", tag="mm512")

            def warmup(n):
                for _ in range(n):
                    nc.tensor.matmul(
                        wps[:], wt[:, 0:128], wt[:, 128:640], start=True,
                        stop=True,
                    )

            # ------- batch-0 input load: x half-tiles on two separate queues
            # (batch 0 is latency-critical; steady-state loads ride sync)
            def load(b):
                s = {"b": b}
                xt = big.tile([128, CT, N], F32, name="xT")
                for ct in range(CT):
                    eng = nc.gpsimd if (b == 0 and ct == 1) else nc.sync
                    eng.dma_start(
                        out=xt[:, ct, :],
                        in_=x_d[b, ct * 128 : (ct + 1) * 128, :],
                    )
                s["x"] = xt
                return s

            cur = load(0)

            # ------- constants: gn smalls on the scalar queue (needed first),
            # big weight tiles on the gpsimd queue
            vec_t = consts.tile([128, 5, CT], F32, name="vec_t")
            nc.scalar.dma_start(out=vec_t[:], in_=vec_d[:, :, :])
            GAM, BET, BQ, BK, BP = range(5)

            g8_t = consts.tile([128, CT, G], F32R, name="g8_t")
            nc.scalar.dma_start(out=g8_t[:], in_=g8_d[:, :, :].bitcast(F32R))
            gt_t = consts.tile([G, CT, 128], F32R, name="gt_t")
            nc.scalar.dma_start(
                out=gt_t[:],
                in_=gt_d[:, :].rearrange("g (ct p) -> g ct p", p=128).bitcast(F32R),
            )
            if USE_DR:
                ones_t = consts.tile([128, 2, 128], FP8, name="ones_t")
                nc.scalar.dma_start(out=ones_t[:], in_=ones_d[:, :, :])
            else:
                ones_t = consts.tile([128, 128], BF16, name="ones_t")
                nc.scalar.dma_start(out=ones_t[:], in_=_bcast_ap(ones_d, 128))

            w_tiles = {}
            wlist = (
                (("wq", wq_d), ("wk", wk_d)) if qk_bias else (("wa", wa_d),)
            ) + (("wv", wv_d), ("wp", wp_d))
            for nm, d in wlist:
                t = consts.tile([128, CT, C], BF16, name=f"{nm}_t")
                nc.gpsimd.dma_start(
                    out=t[:],
                    in_=d[:, :].rearrange("(ci p) o -> p ci o", p=128),
                )
                w_tiles[nm] = t
            wv_t, wp_t = w_tiles["wv"], w_tiles["wp"]

            # ---------------- per-batch stages ----------------

            def gn_stats(s):
                """bn stats -> per-channel [mean, E[x^2]+eps]. DVE-only: no
                PE instruction, so it can be emitted early (the PE queue
                never blocks on it)."""
                xt = s["x"]
                st2s = []
                for ct in range(CT):
                    xin = xt[:, ct, :].rearrange("p (s f) -> p s f", f=512)
                    st6 = small.tile([128, 2, 6], F32, name="st6")
                    for sg in range(2):
                        nc.vector.bn_stats(out=st6[:, sg, :], in_=xin[:, sg, :])
                    mv = small.tile([128, 2], F32, name="mv")
                    nc.vector.bn_aggr(out=mv[:], in_=st6[:])
                    st2 = small.tile([128, 2], F32R, name=f"st2_{ct}")
                    nc.vector.tensor_copy(out=st2[:, 0:1], in_=mv[:, 0:1])
                    sq = small.tile([128, 1], F32, name="sq")
                    nc.vector.tensor_mul(out=sq[:], in0=mv[:, 0:1], in1=mv[:, 0:1])
                    # col1 = E[x^2] + eps  (G8 rows sum to 1, so eps survives)
                    nc.vector.scalar_tensor_tensor(
                        out=st2[:, 1:2], in0=sq[:], scalar=EPS, in1=mv[:, 1:2],
                        op0=OP.add, op1=OP.add,
                    )
                    st2s.append(st2)
                s["st2s"] = st2s

            def gn_group(s):
                """Group-stat matmul + Newton rsqrt -> sg2 = [mean_g, rstd_g].
                Emitted only once the bn-stats chain is close to done: the
                gsp matmul head-of-line blocks the PE queue while waiting."""
                st2s = s["st2s"]
                gsp = ps_m.tile([G, 2], F32, name="gsp", tag="mm512")
                for ci in range(CT):
                    nc.tensor.matmul(
                        gsp[:], g8_t[:, ci, :], st2s[ci][:],
                        start=(ci == 0), stop=(ci == CT - 1),
                    )
                gss = small.tile([G, 2], F32, name="gss")
                nc.vector.tensor_copy(out=gss[:], in_=gsp[:])
                # v = (E[x^2]+eps) - mean^2 ; rstd = rsqrt(v)
                gsq = small.tile([G, 1], F32, name="gsq")
                nc.vector.tensor_mul(out=gsq[:], in0=gss[:, 0:1], in1=gss[:, 0:1])
                gv = small.tile([G, 1], F32, name="gv")
                nc.vector.scalar_tensor_tensor(
                    out=gv[:], in0=gsq[:], scalar=-1.0, in1=gss[:, 1:2],
                    op0=OP.mult, op1=OP.add,
                )
                rc = small.tile([G, 1], F32, name="rc")
                nc.vector.reciprocal(out=rc[:], in_=gv[:])
                r = small.tile([G, 1], F32, name="rn0")
                nc.vector.tensor_scalar_min(r[:], rc[:], 1.0)
                sg2 = small.tile([G, 2], F32R, name="sg2")
                nc.vector.tensor_copy(out=sg2[:, 0:1], in_=gss[:, 0:1])
                for it in range(2):
                    t1 = small.tile([G, 1], F32, name="nw_t1")
                    nc.vector.tensor_mul(out=t1[:], in0=r[:], in1=r[:])
                    t2 = small.tile([G, 1], F32, name="nw_t2")
                    nc.vector.scalar_tensor_tensor(
                        out=t2[:], in0=t1[:], scalar=-0.5, in1=gv[:],
                        op0=OP.mult, op1=OP.mult,
                    )
                    dst = sg2[:, 1:2] if it == 1 else small.tile(
                        [G, 1], F32, name="nw_r"
                    )
                    nc.vector.scalar_tensor_tensor(
                        out=dst, in0=t2[:], scalar=1.5, in1=r[:],
                        op0=OP.add, op1=OP.mult,
                    )
                    if it < 1:
                        r = dst
                s["sg2"] = sg2

            def gn_post(s):
                """Broadcast group stats to channels; per-channel affine
                A = rstd*gamma, B2 = mean*A - beta (h computed as x*A - B2)."""
                a_t = small.tile([128, CT], F32, name="a_vec")
                b2_t = small.tile([128, CT], F32, name="b2_vec")
                for ct in range(CT):
                    csp = ps_m.tile([128, 2], F32, name="csp", tag="mm512")
                    nc.tensor.matmul(
                        csp[:], gt_t[:, ct, :], s["sg2"][:], start=True, stop=True
                    )
                    nc.vector.tensor_mul(
                        out=a_t[:, ct : ct + 1], in0=csp[:, 1:2],
                        in1=vec_t[:, GAM, ct : ct + 1],
                    )
                    nc.vector.scalar_tensor_tensor(
                        out=b2_t[:, ct : ct + 1], in0=csp[:, 0:1],
                        scalar=a_t[:, ct : ct + 1], in1=vec_t[:, BET, ct : ct + 1],
                        op0=OP.mult, op1=OP.subtract,
                    )
                s["a"], s["b2"] = a_t, b2_t
                ht = big.tile([128, CT, N], BF16, name="hT")
                for mch in range(MCH):
                    msl = slice(mch * 512, (mch + 1) * 512)
                    for ct in range(CT):
                        nc.vector.tensor_scalar(
                            ht[:, ct, msl], s["x"][:, ct, msl],
                            a_t[:, ct : ct + 1], b2_t[:, ct : ct + 1],
                            OP.mult, OP.subtract,
                        )
                s["h"] = ht

            def stage_proj(s):
                """q,k / folded-u (natural) and vT (transposed) projections."""
                ht = s["h"]

                if qk_bias:
                    qt = big.tile([128, CT, N], BF16, name="qT")
                    kt = big.tile([128, CT, N], BF16, name="kT")
                    pairs = ((qt, w_tiles["wq"], BQ), (kt, w_tiles["wk"], BK))
                else:
                    # u = wa^T... : s[m,n] = sum_c h[c,m] u[c,n]
                    ut = big.tile([128, CT, N], BF16, name="qT")
                    pairs = ((ut, w_tiles["wa"], None),)
                for dst, w_t, bias_idx in pairs:
                    for co in range(CT):
                        acc = ps_a.tile([128, N], F32, name="acc", tag="acc")
                        for mch in range(MCH):
                            msl = slice(mch * 512, (mch + 1) * 512)
                            for ci in range(CT):
                                nc.tensor.matmul(
                                    acc[:, msl],
                                    w_t[:, ci, co * 128 : (co + 1) * 128],
                                    ht[:, ci, msl],
                                    start=(ci == 0),
                                    stop=(ci == CT - 1),
                                )
                        nc.scalar.activation(
                            out=dst[:, co, :], in_=acc[:], func=AF.Identity,
                            bias=(0.0 if bias_idx is None
                                  else vec_t[:, bias_idx, co : co + 1]),
                            scale=1.0,
                        )
                if qk_bias:
                    s["q"], s["k"] = qt, kt
                else:
                    # sT[n,m] = sum_c u[c,n] h[c,m]: u is stationary, h moving
                    s["q"], s["k"] = ht, ut

                # v computed transposed, stored as fp8 pairs [n128, 2, C]
                # (pair index = adjacent n-tile) for DoubleRow attend.
                vts = []
                for np_ in range(NP):
                    vt = vtp.tile([128, 2, C], P_DT, name=f"vt{np_}")
                    for j in range(2):
                        nt = 2 * np_ + j
                        vp = ps_m.tile([128, C], F32, name="vp", tag="mm512")
                        for ci in range(CT):
                            nc.tensor.matmul(
                                vp[:],
                                ht[:, ci, nt * 128 : (nt + 1) * 128],
                                wv_t[:, ci, :],
                                start=(ci == 0),
                                stop=(ci == CT - 1),
                            )
                        # bv folded into bp' on host: plain convert eviction
                        nc.vector.tensor_copy(out=vt[:, j, :], in_=vp[:])
                    vts.append(vt)
                s["v"] = vts

            def stage_b(s, nxt_b):
                """scores^T -> exp -> pT ; row sums; next batch's load and gn
                chain interleaved so their latency hides under PE work."""
                nxt = None
                rs = ps_rs.tile([128, N], F32, name="rsp")
                pts = []
                for np_ in range(NP):
                    pt = ptp.tile([128, 2, N], P_DT, name=f"pt{np_}")
                    for j in range(2):
                        nt = 2 * np_ + j
                        stp = ps_a.tile([128, N], F32, name="stp", tag="acc")
                        for mch in range(MCH):
                            msl = slice(mch * 512, (mch + 1) * 512)
                            for ci in range(CT):
                                nc.tensor.matmul(
                                    stp[:, msl],
                                    s["k"][:, ci, nt * 128 : (nt + 1) * 128],
                                    s["q"][:, ci, msl],
                                    start=(ci == 0),
                                    stop=(ci == CT - 1),
                                )
                        nc.scalar.activation(
                            out=pt[:, j, :], in_=stp[:], func=AF.Exp,
                            bias=ebias_t[:], scale=SCALE,
                        )
                        if nt == 0 and nxt_b is not None:
                            nxt = load(nxt_b)
                        if nt == 4 and nxt is not None:
                            gn_stats(nxt)
                        if nt == 6 and nxt is not None:
                            gn_group(nxt)
                    # row sums via ones matmul; fp8 DoubleRow does both halves
                    # of the pair (K=256) per call
                    for mch in range(MCH):
                        msl = slice(mch * 512, (mch + 1) * 512)
                        if USE_DR:
                            nc.tensor.matmul(
                                rs[:, msl], ones_t[:, :, :], pt[:, :, msl],
                                start=(np_ == 0), stop=(np_ == NP - 1),
                                perf_mode=DR,
                            )
                        else:
                            for j in range(2):
                                nc.tensor.matmul(
                                    rs[:, msl], ones_t[:], pt[:, j, msl],
                                    start=(np_ == 0 and j == 0),
                                    stop=(np_ == NP - 1 and j == 1),
                                )
                    pts.append(pt)
                s["p"] = pts
                s["rs"] = rs
                return nxt

            def stage_c(s, nxt):
                """1/rowsum; attend (+normalize); next batch's gn_post (PE
                is busy with attend while its chain completes, and the norm
                evictions precede h-affine in the DVE queue); project
                (+bias+residual)."""
                rcp = misc.tile([128, N], F32, name="rcp")
                # per m-half so the first attend eviction isn't gated by the
                # full-width reciprocal latency
                for mch in range(MCH):
                    msl = slice(mch * 512, (mch + 1) * 512)
                    nc.vector.reciprocal_approx_fast(
                        out=rcp[:, msl], in_=s["rs"][:, msl]
                    )

                ont = big.tile([128, CT, N], BF16, name="onT")
                for ct in range(CT):
                    for mch in range(MCH):
                        msl = slice(mch * 512, (mch + 1) * 512)
                        ap_ = ps_m.tile([128, 512], F32, name="attp", tag="mm512")
                        for np_ in range(NP):
                            if USE_DR:
                                nc.tensor.matmul(
                                    ap_[:],
                                    s["v"][np_][:, :, ct * 128 : (ct + 1) * 128],
                                    s["p"][np_][:, :, msl],
                                    start=(np_ == 0),
                                    stop=(np_ == NP - 1),
                                    perf_mode=DR,
                                )
                            else:
                                for j in range(2):
                                    nc.tensor.matmul(
                                        ap_[:],
                                        s["v"][np_][:, j,
                                                    ct * 128 : (ct + 1) * 128],
                                        s["p"][np_][:, j, msl],
                                        start=(np_ == 0 and j == 0),
                                        stop=(np_ == NP - 1 and j == 1),
                                    )
                        nc.vector.tensor_mul(
                            out=ont[:, ct, msl], in0=ap_[:], in1=rcp[:, msl]
                        )

                if nxt is not None:
                    gn_post(nxt)

                for co in range(CT):
                    for mch in range(MCH):
                        msl = slice(mch * 512, (mch + 1) * 512)
                        pp = ps_m.tile([128, 512], F32, name="pp", tag="mm512")
                        for ci in range(CT):
                            nc.tensor.matmul(
                                pp[:],
                                wp_t[:, ci, co * 128 : (co + 1) * 128],
                                ont[:, ci, msl],
                                start=(ci == 0),
                                stop=(ci == CT - 1),
                            )
                        outf = misc.tile([128, 512], F32, name="outf")
                        nc.vector.scalar_tensor_tensor(
                            out=outf[:],
                            in0=pp[:],
                            scalar=vec_t[:, BP, co : co + 1],
                            in1=s["x"][:, co, msl],
                            op0=OP.add,
                            op1=OP.add,
                        )
                        eng = nc.scalar if (co + mch) % 2 == 0 else nc.gpsimd
                        eng.dma_start(
                            out=out_d[s["b"], co * 128 : (co + 1) * 128, msl],
                            in_=outf[:],
                        )

            # ---------------- emission schedule ----------------
            # Batch-0 prologue: warm-up matmul groups bracket the GroupNorm
            # emission so the PE queue stays busy (HAM warm) through the
            # serial DVE chain, and the group-stat matmuls are reached only
            # once their inputs are about ready.
            warmup(12)
            gn_stats(cur)
            warmup(6)
            gn_group(cur)
            warmup(4)
            gn_post(cur)
            warmup(4)
            stage_proj(cur)
            for b in range(B_LOC):
                nxt = stage_b(cur, b + 1 if b + 1 < B_LOC else None)
                stage_c(cur, nxt)
                cur = nxt
                if cur is not None:
                    stage_proj(cur)

    nc.finalize()
    return nc


_NC = {}


def _get_nc(qk_bias):
    if qk_bias not in _NC:
        _NC[qk_bias] = _build_nc(qk_bias=qk_bias)
    return _NC[qk_bias]


def _make_in_maps(inputs, qk_bias):
    BF = ml_dtypes.bfloat16
    E4 = ml_dtypes.float8_e4m3
    x = np.asarray(inputs["x"], dtype=np.float32).reshape(B, C, N)
    g8p = np.zeros((128, CT, G), np.float32)
    for c in range(C):
        g8p[c % 128, c // 128, c // 8] = 0.125
    gt = np.zeros((G, C), np.float32)
    for c in range(C):
        gt[c // 8, c] = 1.0
    wp64 = np.asarray(inputs["wp"], np.float64)
    bpp = wp64 @ np.asarray(inputs["bv"], np.float64) + np.asarray(
        inputs["bp"], np.float64
    )  # bv folded: out = x + wp@(o - bv-part) ... wp@bv + bp
    vecs = np.stack(
        [
            np.asarray(inputs["gamma"], np.float32),
            np.asarray(inputs["beta"], np.float32),
            np.asarray(inputs["bq"], np.float32),
            np.asarray(inputs["bk"], np.float32),
            bpp.astype(np.float32),
        ]
    )  # [5, 256]
    vecp = np.ascontiguousarray(
        vecs.reshape(5, CT, 128).transpose(2, 0, 1)
    )  # [128, 5, CT]

    shared = {
        "wvT": np.ascontiguousarray(np.asarray(inputs["wv"], np.float32).T)
        .astype(BF),
        "wpT": np.ascontiguousarray(wp64.T.astype(np.float32)).astype(BF),
        "vecp": vecp,
        "g8p": g8p,
        "gt": gt,
    }
    if USE_DR:
        shared["ones8"] = np.ones((128, 2, 128), np.float32).astype(E4)
    else:
        shared["ones"] = np.ones((128,), np.float32)
    if qk_bias:
        shared["wqT"] = (
            np.ascontiguousarray(np.asarray(inputs["wq"], np.float32).T)
            .astype(BF)
        )
        shared["wkT"] = (
            np.ascontiguousarray(np.asarray(inputs["wk"], np.float32).T)
            .astype(BF)
        )
    else:
        wa = np.asarray(inputs["wk"], np.float64).T @ np.asarray(
            inputs["wq"], np.float64
        )
        shared["waT"] = np.ascontiguousarray(wa.astype(np.float32)).astype(BF)
    in_maps = []
    for i in range(N_CORES):
        m = dict(shared)
        m["x"] = np.ascontiguousarray(x[i * B_LOC : (i + 1) * B_LOC])
        in_maps.append(m)
    return in_maps


def _run(inputs, trace=False):
    from concourse.bass_utils import run_bass_kernel_spmd

    qk_bias = bool(
        np.any(np.asarray(inputs["bq"])) or np.any(np.asarray(inputs["bk"]))
    )
    nc = _get_nc(qk_bias)
    in_maps = _make_in_maps(inputs, qk_bias)
    res = run_bass_kernel_spmd(
        nc, in_maps, core_ids=list(range(N_CORES)), trace=trace
    )
    out = np.concatenate([r["out"] for r in res.results], axis=0)
    return out.reshape(B, C, 32, 32).astype(np.float32), res


def kernel(**inputs) -> np.ndarray:
    out, _ = _run(inputs, trace=False)
    return out
